# revision 2
# baseline (speedup 1.0000x reference)
"""Trainium2 Bass kernel for a 3-layer GAT (PyG GATConv, concat=False) +
global mean pool + linear head, SPMD across 8 NeuronCores.

Strategy:
  - Nodes partitioned across cores at graph boundaries (batch is sorted), so
    segment-softmax, scatter and mean-pool stay core-local.
  - Per layer: node transform replicated on every core via one matmul with
    host-augmented weights [W | W@A_src | W@A_dst] producing a DRAM node
    table [h(256) | a_src-dot(4) | a_dst-dot(4)] for all nodes.
  - Edge phase: per 128-edge tile, indirect-DMA gather of src rows, on-chip
    one-hot matrices (built with iota/is_equal) turn segment softmax +
    scatter-add into PE matmuls accumulated in PSUM.
  - Softmax uses no max-subtraction (mathematically identical ratios; logits
    are O(10) so fp32 exp is safe).
  - Between layers, transposed activations are AllGathered (the only
    collective).
"""
import sys
sys.path.insert(0, '/opt/trn_rl_repo')
import numpy as np

NCORES = 8
H, C, HC = 4, 64, 256
TW = HC + 2 * H      # 264
P = 128
GPC = 32             # graphs per core (256/8)
NEG = 0.2
G = 256


# ---------------------------------------------------------------- wait split
def _split_waits(nc):
    """This walrus build supports only ONE sem-wait per instruction; split
    extra waits onto preceding same-engine NoOps (engines execute in order)."""
    import concourse.mybir as mybir
    for fn in nc.m.functions:
        for bb in fn.blocks:
            new_insts = []
            for inst in bb.instructions:
                si = inst.sync_info
                if si is not None and si.on_wait and len(si.on_wait) > 1:
                    waits = list(si.on_wait)
                    for ci, w in enumerate(waits[:-1]):
                        new_insts.append(mybir.InstNoOp(
                            name=f"{inst.name}-ws{ci}", engine=inst.engine,
                            ins=[], outs=[],
                            sync_info=mybir.SyncInfo(on_wait=[w], on_update=[]),
                            text_hint="waitsplit"))
                    inst.sync_info = mybir.SyncInfo(
                        on_wait=[waits[-1]], on_update=list(si.on_update))
                new_insts.append(inst)
            bb.instructions[:] = new_insts


# ---------------------------------------------------------------- runner
def _make_runner(nc):
    import jax
    from jax.sharding import Mesh, PartitionSpec
    from jax.experimental.shard_map import shard_map
    from concourse.bass2jax import (_bass_exec_p, partition_id_tensor,
                                    install_neuronx_cc_hook)
    import concourse.mybir as mb

    install_neuronx_cc_hook()
    partition_name = nc.partition_id_tensor.name if nc.partition_id_tensor else None

    in_names, out_names, out_avals, zero_outs = [], [], [], []
    for alloc in nc.m.functions[0].allocations:
        if not isinstance(alloc, mb.MemoryLocationSet):
            continue
        name = alloc.memorylocations[0].name
        if alloc.kind == "ExternalInput":
            if name != partition_name:
                in_names.append(name)
        elif alloc.kind == "ExternalOutput":
            shape = tuple(alloc.tensor_shape)
            dtype = mb.dt.np(alloc.dtype)
            out_names.append(name)
            out_avals.append(jax.core.ShapedArray(shape, dtype))
            zero_outs.append(np.zeros(shape, dtype))
    n_params = len(in_names)
    n_outs = len(out_avals)
    all_in = list(in_names) + list(out_names)
    if partition_name is not None:
        all_in.append(partition_name)
    donate = tuple(range(n_params, n_params + n_outs))

    def _body(*args):
        operands = list(args)
        if partition_name is not None:
            operands.append(partition_id_tensor())
        outs = _bass_exec_p.bind(
            *operands, out_avals=tuple(out_avals), in_names=tuple(all_in),
            out_names=tuple(out_names), lowering_input_output_aliases=(),
            sim_require_finite=False, sim_require_nnan=False, nc=nc)
        return tuple(outs)

    devices = jax.devices()[:NCORES]
    mesh = Mesh(np.asarray(devices), ("core",))
    sharded = jax.jit(
        shard_map(_body, mesh=mesh,
                  in_specs=(PartitionSpec("core"),) * (n_params + n_outs),
                  out_specs=(PartitionSpec("core"),) * n_outs,
                  check_rep=False),
        donate_argnums=donate, keep_unused=True)

    def run(in_maps):
        concat_in = [
            np.concatenate([np.asarray(in_maps[c][nm]) for c in range(NCORES)], axis=0)
            for nm in in_names]
        zeros = [np.zeros((NCORES * z.shape[0], *z.shape[1:]), z.dtype)
                 for z in zero_outs]
        out_arrs = sharded(*concat_in, *zeros)
        out_arrs = [np.asarray(o) for o in out_arrs]
        return [
            {nm: out_arrs[i].reshape(NCORES, *out_avals[i].shape)[c]
             for i, nm in enumerate(out_names)}
            for c in range(NCORES)]

    return run


# ---------------------------------------------------------------- preprocessing
def _make_A(att):
    A = np.zeros((HC, H), np.float32)
    for h in range(H):
        A[h * C:(h + 1) * C, h] = att[h]
    return A


def _preprocess(inputs):
    x = np.asarray(inputs["x"], np.float32)
    ei = np.asarray(inputs["edge_index"])
    batch = np.asarray(inputs["batch"]).astype(np.int64)
    N = x.shape[0]

    gstart = np.searchsorted(batch, np.arange(G + 1))
    node_start = gstart[np.arange(NCORES) * GPC]
    nloc = np.append(node_start[1:], N) - node_start
    NL = int(np.ceil(nloc.max() / P) * P)
    B = NL // P
    NTOT = NCORES * NL

    owner = np.searchsorted(node_start, np.arange(N), side="right") - 1
    pad_id = (owner * NL + (np.arange(N) - node_start[owner])).astype(np.int64)

    src = np.concatenate([ei[0], np.arange(N, dtype=np.int64)]).astype(np.int64)
    dst = np.concatenate([ei[1], np.arange(N, dtype=np.int64)]).astype(np.int64)
    e_owner = owner[dst]
    e_li = dst - node_start[e_owner]
    e_blk = e_li // P
    e_rel = e_li % P

    counts = np.zeros((NCORES, B), np.int64)
    np.add.at(counts, (e_owner, e_blk), 1)
    K = int(np.ceil(counts.max() / P))

    order = np.lexsort((e_blk, e_owner))
    so, sb, srel, ssrc = e_owner[order], e_blk[order], e_rel[order], src[order]
    key = so * B + sb
    grp_starts = np.searchsorted(key, np.arange(NCORES * B))
    pos = np.arange(len(key)) - grp_starts[key]
    k_of, p_of = pos // P, pos % P
    col = sb * K + k_of
    src_idx = np.zeros((NCORES, P, B * K), np.int32)
    dst_rel = np.full((NCORES, P, B * K), -1.0, np.float32)
    src_idx[so, p_of, col] = pad_id[ssrc]
    dst_rel[so, p_of, col] = srel.astype(np.float32)

    # sort each 128-edge tile by src row for HBM locality (S handles any order)
    perm = np.argsort(np.where(dst_rel < 0, np.int32(-1), src_idx), axis=1,
                      kind="stable")
    src_idx = np.take_along_axis(src_idx, perm, axis=1)
    dst_rel = np.take_along_axis(dst_rel, perm, axis=1)

    adidx = np.zeros((NCORES, P, B), np.int32)
    for c_ in range(NCORES):
        adidx[c_] = (c_ * NL + np.arange(B)[None, :] * P +
                     np.arange(P)[:, None]).astype(np.int32)

    batchrel = np.full((NCORES, P, B), -1.0, np.float32)
    for c_ in range(NCORES):
        nn = nloc[c_]
        vals = (batch[node_start[c_]:node_start[c_] + nn] - c_ * GPC).astype(np.float32)
        li = np.arange(nn)
        batchrel[c_, li % P, li // P] = vals

    cnt = np.zeros(G, np.float32)
    np.add.at(cnt, batch, 1.0)
    recip = (1.0 / np.maximum(cnt, 1.0)).astype(np.float32)
    recip_pc = recip.reshape(NCORES, GPC, 1).copy()

    def aug(Wn, a_s, a_d):
        W = np.asarray(inputs[Wn], np.float32)
        return np.ascontiguousarray(np.concatenate(
            [W, W @ _make_A(np.asarray(inputs[a_s], np.float32)),
             W @ _make_A(np.asarray(inputs[a_d], np.float32))], axis=1))

    IN = x.shape[1]
    x1T = np.zeros((IN, NTOT), np.float32)
    x1T[:, pad_id] = x.T

    bias_rep = np.stack([
        np.broadcast_to(np.asarray(inputs[b], np.float32), (P, C)).copy()
        for b in ("b1", "b2", "b3")])                     # [3, P, C]

    return dict(
        NL=NL, B=B, K=K, NTOT=NTOT,
        src_idx=src_idx, dst_rel=dst_rel, adidx=adidx, batchrel=batchrel,
        recip_pc=recip_pc,
        Wa1=aug("W1", "att_src1", "att_dst1"),
        Wa2=aug("W2", "att_src2", "att_dst2"),
        Wa3=aug("W3", "att_src3", "att_dst3"),
        x1T=x1T, bias_rep=bias_rep,
        lin_w=np.asarray(inputs["lin_w"], np.float32).reshape(C, 1),
        linb_rep=np.full((GPC, 1), float(np.asarray(inputs["lin_b"]).reshape(-1)[0]),
                         np.float32),
        ident=np.eye(P, dtype=np.float32),
        iota=np.broadcast_to(np.arange(P, dtype=np.float32), (P, P)).copy(),
        pcol=np.arange(P, dtype=np.float32).reshape(P, 1),
    )


# ---------------------------------------------------------------- bass builder
def _build(NL, B, K):
    import concourse.bass as bass
    import concourse.mybir as mybir
    from concourse.tile import TileContext

    NTOT = NCORES * NL
    f32 = mybir.dt.float32
    i32 = mybir.dt.int32
    Alu = mybir.AluOpType
    Act = mybir.ActivationFunctionType

    nc = bass.Bass()
    # inputs
    x1T = nc.dram_tensor("x1T", [P, NTOT], f32, kind="ExternalInput")
    Wa = [nc.dram_tensor(f"Wa{l+1}", [P if l == 0 else C, TW], f32,
                         kind="ExternalInput") for l in range(3)]
    bias_rep = nc.dram_tensor("bias_rep", [3, P, C], f32, kind="ExternalInput")
    srcidx = nc.dram_tensor("srcidx", [P, B * K], i32, kind="ExternalInput")
    dstrel = nc.dram_tensor("dstrel", [P, B * K], f32, kind="ExternalInput")
    adidx = nc.dram_tensor("adidx", [P, B], i32, kind="ExternalInput")
    batchrel = nc.dram_tensor("batchrel", [P, B], f32, kind="ExternalInput")
    recip = nc.dram_tensor("recip", [GPC, 1], f32, kind="ExternalInput")
    linw = nc.dram_tensor("linw", [C, 1], f32, kind="ExternalInput")
    linb_rep = nc.dram_tensor("linb_rep", [GPC, 1], f32, kind="ExternalInput")
    ident_in = nc.dram_tensor("ident", [P, P], f32, kind="ExternalInput")
    iota_in = nc.dram_tensor("iota", [P, P], f32, kind="ExternalInput")
    pcol_in = nc.dram_tensor("pcol", [P, 1], f32, kind="ExternalInput")
    # output
    y = nc.dram_tensor("y", [GPC, 1], f32, kind="ExternalOutput")
    # internal
    table = nc.dram_tensor("table", [NTOT, TW], f32)
    agin = [nc.dram_tensor(f"agin{l}", [C, NL], f32) for l in range(2)]
    agout = [nc.dram_tensor(f"agout{l}", [NCORES * C, NL], f32,
                            addr_space="Shared") for l in range(2)]

    CHT = max(d for d in range(1, min(10, B) + 1) if B % d == 0)
    CHW = CHT * P             # transform x-chunk columns
    NCH = (B * P) // CHW      # chunks per section

    with TileContext(nc) as tc:
        with (
            tc.tile_pool(name="const", bufs=1) as cpool,
            tc.tile_pool(name="xch", bufs=3) as xpool,
            tc.tile_pool(name="g", bufs=3) as gpool,
            tc.tile_pool(name="row", bufs=4) as rowpool,
            tc.tile_pool(name="s", bufs=4) as spool,
            tc.tile_pool(name="small", bufs=6) as lpool,
            tc.tile_pool(name="o", bufs=4) as opool,
            tc.tile_pool(name="w", bufs=2) as wpool,
            tc.tile_pool(name="ps", bufs=2, space="PSUM") as psum,
            tc.tile_pool(name="pspool", bufs=1, space="PSUM") as pspool,
        ):
            # ---- constants
            ident_sb = cpool.tile([P, P], f32)
            nc.sync.dma_start(out=ident_sb[:], in_=ident_in[:])
            iota_sb = cpool.tile([P, P], f32)
            nc.sync.dma_start(out=iota_sb[:], in_=iota_in[:])
            pcol_sb = cpool.tile([P, 1], f32)
            nc.sync.dma_start(out=pcol_sb[:], in_=pcol_in[:])
            srcidx_sb = cpool.tile([P, B * K], i32)
            nc.sync.dma_start(out=srcidx_sb[:], in_=srcidx[:])
            dstrel_sb = cpool.tile([P, B * K], f32)
            nc.sync.dma_start(out=dstrel_sb[:], in_=dstrel[:])
            adidx_sb = cpool.tile([P, B], i32)
            nc.sync.dma_start(out=adidx_sb[:], in_=adidx[:])
            batchrel_sb = cpool.tile([P, B], f32)
            nc.sync.dma_start(out=batchrel_sb[:], in_=batchrel[:])
            recip_sb = cpool.tile([GPC, 1], f32)
            nc.sync.dma_start(out=recip_sb[:], in_=recip[:])
            linw_sb = cpool.tile([C, 1], f32)
            nc.sync.dma_start(out=linw_sb[:], in_=linw[:])
            linb_sb = cpool.tile([GPC, 1], f32)
            nc.sync.dma_start(out=linb_sb[:], in_=linb_rep[:])

            pool_ps = pspool.tile([GPC, C], f32, tag="pool")

            for l in range(3):
                KIN = P if l == 0 else C
                # ---------------- transform: table[n] = xT[:,n].T @ Wa_l
                Wsb = wpool.tile([KIN, TW], f32, tag="W")
                nc.sync.dma_start(out=Wsb[:], in_=Wa[l][:])
                brep_sb = wpool.tile([P, C], f32, tag="brep")
                nc.sync.dma_start(out=brep_sb[:], in_=bias_rep[l])
                for s in range(NCORES):
                    for ch in range(NCH):
                        c0 = ch * CHW
                        xchunk = xpool.tile([KIN, CHW], f32, tag="xch")
                        if l == 0:
                            nc.sync.dma_start(
                                out=xchunk[:],
                                in_=x1T[:, s * NL + c0:s * NL + c0 + CHW])
                        else:
                            nc.sync.dma_start(
                                out=xchunk[:],
                                in_=agout[l - 1][s * C:(s + 1) * C,
                                                 c0:c0 + CHW])
                        for t in range(CHT):
                            ps_t = psum.tile([P, TW], f32, tag="big")
                            nc.tensor.matmul(
                                out=ps_t[:],
                                lhsT=xchunk[:, t * P:(t + 1) * P],
                                rhs=Wsb[:], start=True, stop=True)
                            row = rowpool.tile([P, TW], f32, tag="row")
                            nc.scalar.copy(out=row[:], in_=ps_t[:])
                            r0 = s * NL + c0 + t * P
                            nc.sync.dma_start(out=table[r0:r0 + P, :],
                                              in_=row[:])

                # ---------------- edge phase
                adg = wpool.tile([P, B * H], f32, tag="adg")
                for b in range(B):
                    nc.gpsimd.indirect_dma_start(
                        out=adg[:, b * H:(b + 1) * H], out_offset=None,
                        in_=table[:],
                        in_offset=bass.IndirectOffsetOnAxis(
                            ap=adidx_sb[:, b:b + 1], axis=0),
                        element_offset=HC + H)
                for b in range(B):
                    g = gpool.tile([P, K * TW], f32, tag="g")
                    for k in range(K):
                        cix = b * K + k
                        nc.gpsimd.indirect_dma_start(
                            out=g[:, k * TW:(k + 1) * TW], out_offset=None,
                            in_=table[:],
                            in_offset=bass.IndirectOffsetOnAxis(
                                ap=srcidx_sb[:, cix:cix + 1], axis=0))
                    ns = psum.tile([P, HC + H], f32, tag="big")
                    for k in range(K):
                        cix = b * K + k
                        drc = dstrel_sb[:, cix:cix + 1]
                        S = spool.tile([P, P], f32, tag="S")
                        nc.vector.tensor_tensor(
                            out=S[:], in0=drc.to_broadcast([P, P]),
                            in1=iota_sb[:], op=Alu.is_equal)
                        dT = psum.tile([P, P], f32, tag="mid")
                        nc.tensor.transpose(
                            out=dT[:], in_=drc.to_broadcast([P, P]),
                            identity=ident_sb[:])
                        ST = spool.tile([P, P], f32, tag="ST")
                        nc.vector.tensor_tensor(
                            out=ST[:], in0=pcol_sb[:].to_broadcast([P, P]),
                            in1=dT[:], op=Alu.is_equal)
                        ade = psum.tile([P, H], f32, tag="small")
                        nc.tensor.matmul(
                            out=ade[:], lhsT=ST[:],
                            rhs=adg[:, b * H:(b + 1) * H],
                            start=True, stop=True)
                        we = rowpool.tile([P, HC + H], f32, tag="row")
                        lg = lpool.tile([P, H], f32, tag="lg")
                        nc.vector.tensor_tensor(
                            out=lg[:], in0=g[:, k * TW + HC:k * TW + HC + H],
                            in1=ade[:], op=Alu.add)
                        lm = lpool.tile([P, H], f32, tag="lm")
                        nc.vector.tensor_scalar(
                            out=lm[:], in0=lg[:], scalar1=NEG, scalar2=None,
                            op0=Alu.mult)
                        nc.vector.tensor_tensor(
                            out=lm[:], in0=lg[:], in1=lm[:], op=Alu.max)
                        nc.scalar.activation(
                            out=we[:, HC:HC + H], in_=lm[:], func=Act.Exp)
                        # w[:, h*64+c] = g_h[c] * e_h  (stride-0 bcast on in1)
                        nc.vector.tensor_tensor(
                            out=we[:, 0:HC].rearrange("p (h c) -> p h c", h=H),
                            in0=g[:, k * TW:k * TW + HC].rearrange(
                                "p (h c) -> p h c", h=H),
                            in1=we[:, HC:HC + H].to_broadcast([P, H, C]),
                            op=Alu.mult)
                        nc.tensor.matmul(
                            out=ns[:], lhsT=S[:], rhs=we[:],
                            start=(k == 0), stop=(k == K - 1))
                    # block epilogue
                    s4 = lpool.tile([P, H], f32, tag="s4")
                    nc.vector.tensor_scalar(
                        out=s4[:], in0=ns[:, HC:HC + H], scalar1=1e-30,
                        scalar2=None, op0=Alu.max)
                    r4 = lpool.tile([P, H], f32, tag="r4")
                    nc.vector.reciprocal(out=r4[:], in_=s4[:])
                    nc.vector.tensor_scalar(
                        out=r4[:], in0=r4[:], scalar1=0.25, scalar2=None,
                        op0=Alu.mult)
                    o = opool.tile([P, C], f32, tag="o")
                    tmp = opool.tile([P, C], f32, tag="tmp")
                    nc.vector.tensor_scalar(
                        out=o[:], in0=ns[:, 0:C], scalar1=r4[:, 0:1],
                        scalar2=None, op0=Alu.mult)
                    for h in range(1, H):
                        nc.vector.tensor_scalar(
                            out=tmp[:], in0=ns[:, h * C:(h + 1) * C],
                            scalar1=r4[:, h:h + 1], scalar2=None, op0=Alu.mult)
                        nc.vector.tensor_tensor(
                            out=o[:], in0=o[:], in1=tmp[:], op=Alu.add)
                    nc.vector.tensor_tensor(
                        out=o[:], in0=o[:], in1=brep_sb[:], op=Alu.add)
                    nc.scalar.activation(out=o[:], in_=o[:], func=Act.Relu)
                    if l < 2:
                        oT = psum.tile([C, P], f32, tag="mid")
                        nc.tensor.transpose(out=oT[:], in_=o[:],
                                            identity=ident_sb[:])
                        ag = opool.tile([C, P], f32, tag="ag")
                        nc.scalar.copy(out=ag[:], in_=oT[:])
                        nc.sync.dma_start(
                            out=agin[l][:, b * P:(b + 1) * P], in_=ag[:])
                    else:
                        Sg = spool.tile([P, GPC], f32, tag="Sg")
                        nc.vector.tensor_tensor(
                            out=Sg[:], in0=batchrel_sb[:, b:b + 1].to_broadcast(
                                [P, GPC]),
                            in1=iota_sb[:, 0:GPC], op=Alu.is_equal)
                        nc.tensor.matmul(
                            out=pool_ps[:], lhsT=Sg[:], rhs=o[:],
                            start=(b == 0), stop=(b == B - 1))
                if l < 2:
                    nc.gpsimd.collective_compute(
                        "AllGather", Alu.bypass,
                        ins=[agin[l][:]], outs=[agout[l][:]],
                        replica_groups=[list(range(NCORES))])

            # ---------------- final head
            poolsb = opool.tile([GPC, C], f32, tag="poolsb")
            nc.vector.tensor_scalar(
                out=poolsb[:], in0=pool_ps[:], scalar1=recip_sb[:, 0:1],
                scalar2=None, op0=Alu.mult)
            pT = psum.tile([C, GPC], f32, tag="mid")
            nc.tensor.transpose(out=pT[:], in_=poolsb[:],
                                identity=ident_sb[:GPC, :GPC])
            pTs = opool.tile([C, GPC], f32, tag="pTs")
            nc.scalar.copy(out=pTs[:], in_=pT[:])
            yps = psum.tile([GPC, 1], f32, tag="small")
            nc.tensor.matmul(out=yps[:], lhsT=pTs[:], rhs=linw_sb[:],
                             start=True, stop=True)
            ysb = opool.tile([GPC, 1], f32, tag="ysb")
            nc.vector.tensor_tensor(out=ysb[:], in0=yps[:], in1=linb_sb[:],
                                    op=Alu.add)
            nc.sync.dma_start(out=y[:], in_=ysb[:])

    _split_waits(nc)
    return nc


# ---------------------------------------------------------------- entry point
_CACHE = {}


def kernel(**inputs):
    pp = _preprocess(inputs)
    key = (pp["NL"], pp["B"], pp["K"])
    if key not in _CACHE:
        nc = _build(*key)
        _CACHE[key] = _make_runner(nc)
    run = _CACHE[key]

    shared = dict(
        x1T=pp["x1T"], Wa1=pp["Wa1"], Wa2=pp["Wa2"], Wa3=pp["Wa3"],
        bias_rep=pp["bias_rep"], linw=pp["lin_w"],
        ident=pp["ident"], iota=pp["iota"], pcol=pp["pcol"],
        linb_rep=pp["linb_rep"])
    in_maps = []
    for c in range(NCORES):
        m = dict(shared)
        m.update(srcidx=pp["src_idx"][c], dstrel=pp["dst_rel"][c],
                 adidx=pp["adidx"][c], batchrel=pp["batchrel"][c],
                 recip=pp["recip_pc"][c])
        in_maps.append(m)
    res = run(in_maps)
    return np.concatenate([res[c]["y"] for c in range(NCORES)], axis=0)
